# revision 23
# baseline (speedup 1.0000x reference)
"""Multi-head attention (B=1, S=4096, D=768, H=12) on 8 Trainium2 NeuronCores.

Sharding: 4 head-groups x 2 sequence-halves. Core (g, s) computes heads
[3g, 3g+3) for query rows [2048*s, 2048*(s+1)): it projects q for its rows,
k/v for its heads over the full sequence, runs softmax(QK^T/8)V for its
(heads, rows) block, and applies its slice of the output projection. The
o-proj partials of the 4 head-groups are summed on the host (the all-reduce
step of tensor-parallel attention), halves concatenated, bias added.

On-chip layout notes:
 - scores are built transposed ([keys, queries]) so the attn@V matmul can
   contract keys on the partition axis with no transposes anywhere.
 - the head pair (h0, h1) shares the 128-row PE array via row tiling
   (K=64 each); the odd head h2 runs in rows 0-63 alone.
 - exp row-sums come for free from the attn@V matmul: V is extended with a
   65th column of ones, so PSUM row 64 accumulates sum_k exp(score).
 - softmax uses no max-subtraction: |scores| < ~30 here, safe in fp32.
"""

import numpy as np
import ml_dtypes

import concourse.bass as bass
import concourse.mybir as mybir
import concourse.tile as tile

BF16 = mybir.dt.bfloat16
FP32 = mybir.dt.float32

D = 768            # model dim
HD = 64            # head dim
HPC = 3            # heads per core
DH = HPC * HD      # 192: head dims per core
SEQ = 4096         # full sequence (keys)
SQ = 2048          # query rows per core
CT = D // 128      # 6 contraction tiles for projections
QB = 512           # query block (matmul free dim)
NQB = SQ // QB     # 4
KBLK = 128         # key block (PSUM partition dim)
NKB = SEQ // KBLK  # 32
KT = 512           # k/v load superblock
NKT = SEQ // KT    # 8
SCALE = 1.0 / 8.0  # 1/sqrt(HD)


def _patch_tile_drain():
    """walrus here accepts only one sync-wait per CTRL instruction; the stock
    TileContext exit packs every outstanding wait onto a single SP Drain.
    Split them onto single-wait SP NOPs that precede the drain."""
    import bass_rust
    from concourse.vector_clock import ScopedClock

    def _split_drain_and_barrier(self, tick_clock, wait_clock):
        nc = self.nc
        probe = nc.sync.nop(nofuse=True)
        wait_clock.add_sem_waits(
            probe.ins, ScopedClock({None: tick_clock.global_clock})
        )
        si = probe.ins.sync_info
        waits = list(si.on_wait) if si is not None and si.on_wait else []
        if len(waits) > 1:
            probe.ins.sync_info = bass_rust.SyncInfo(
                on_wait=[waits[0]], on_update=[]
            )
            for w in waits[1:]:
                n = nc.sync.nop(nofuse=True)
                n.ins.sync_info = bass_rust.SyncInfo(on_wait=[w], on_update=[])
        nc.sync.drain()
        nc.all_engine_barrier()
        assert self.sems is not None
        popped = nc._tile_sem_poison_stack.pop()
        assert popped is self._sem_poison
        nc.clear_and_free_semaphores(list(self.sems.allocated().values()))
        nc.all_engine_barrier()

    tile.TileContext._drain_and_barrier = _split_drain_and_barrier



def _split_multi_waits(nc):
    """Hoist all-but-one sync-waits of every instruction onto preceding
    single-wait NOPs on the same engine (walrus 1-wait limit)."""
    import bass_rust
    n_split = 0
    for bb in nc.main_func.blocks:
        insts = bb.instructions
        new_list = []
        for inst in insts:
            si = getattr(inst, "sync_info", None)
            if si is not None and si.on_wait and len(si.on_wait) > 1:
                waits = list(si.on_wait)
                n_split += 1
                for w in waits[:-1]:
                    nop = mybir.InstNoOp(
                        name=nc.get_next_instruction_name(),
                        engine=inst.engine, ins=[], outs=[],
                        sync_info=bass_rust.SyncInfo(
                            on_wait=[w], on_update=[]))
                    new_list.append(nop)
                inst.sync_info = bass_rust.SyncInfo(
                    on_wait=[waits[-1]], on_update=list(si.on_update))
            new_list.append(inst)
        insts[:] = new_list
    return n_split

def build_program(has_bq: bool, has_bk: bool, has_bv: bool,
                  repeat: int = 1, qk_dtype=BF16) -> bass.Bass:
    _patch_tile_drain()
    nc = bass.Bass()

    qTs = nc.dram_tensor("qTs", [D, SQ], BF16, kind="ExternalInput")
    kT = nc.dram_tensor("kT", [D, SEQ], BF16, kind="ExternalInput")
    vT = nc.dram_tensor("vT", [D, SEQ], BF16, kind="ExternalInput")
    wq = nc.dram_tensor("wq", [D, DH], BF16, kind="ExternalInput")
    wk = nc.dram_tensor("wk", [D, DH], BF16, kind="ExternalInput")
    wv = nc.dram_tensor("wv", [D, DH], BF16, kind="ExternalInput")
    wo = nc.dram_tensor("wo", [DH, D], BF16, kind="ExternalInput")
    bqd = nc.dram_tensor("bq", [DH, 1], FP32, kind="ExternalInput")
    bkd = nc.dram_tensor("bk", [DH, 1], FP32, kind="ExternalInput")
    bvd = nc.dram_tensor("bv", [DH, 1], FP32, kind="ExternalInput")
    outT = nc.dram_tensor("outT", [D, SQ], FP32, kind="ExternalOutput")

    Exp = mybir.ActivationFunctionType.Exp

    with tile.TileContext(nc) as tc:
        with (
            tc.tile_pool(name="persist", bufs=1) as persist,
            tc.tile_pool(name="small", bufs=2) as small,
        ):
            # persistent SBUF tensors
            khT_pair = persist.tile([128, SEQ], qk_dtype, tag="khp", name="khp")
            khT_h2 = persist.tile([64, SEQ], qk_dtype, tag="kh2", name="kh2")
            qhT_pair = persist.tile([128, SQ], qk_dtype, tag="qhp", name="qhp")
            qhT_h2 = persist.tile([64, SQ], qk_dtype, tag="qh2", name="qh2")
            vhx = [persist.tile([128, NKB * 65], BF16, tag=f"vhx{h}", name=f"vhx{h}")
                   for h in range(HPC)]
            wq_sb = persist.tile([128, CT * DH], BF16, tag="wq", name="wq_sb")
            wk_sb = persist.tile([128, CT * DH], BF16, tag="wk", name="wk_sb")
            wv_sb = persist.tile([128, CT * DH], BF16, tag="wv", name="wv_sb")
            wo_sb1 = persist.tile([128, D], BF16, tag="wo1", name="wo1")
            wo_sb2 = persist.tile([64, D], BF16, tag="wo2", name="wo2")
            bq_sb = persist.tile([128, 1], FP32, tag="bq1", name="bq1")
            bq2_sb = persist.tile([64, 1], FP32, tag="bq2", name="bq2")
            bk_sb = persist.tile([128, 1], FP32, tag="bk1", name="bk1")
            bk2_sb = persist.tile([64, 1], FP32, tag="bk2", name="bk2")
            bv_sb = persist.tile([64, HPC], FP32, tag="bv", name="bv_sb")
            ones_sb = persist.tile([1, 64], FP32, tag="ones", name="ones_sb")

            # ones columns for the exp-sum trick (overwritten with vh below)
            for h in range(HPC):
                nc.gpsimd.memset(vhx[h][:], 1.0)
            nc.vector.memset(ones_sb[:], 1.0)

            persist_tiles = (khT_pair, khT_h2, qhT_pair, qhT_h2, vhx,
                             wq_sb, wk_sb, wv_sb, wo_sb1, wo_sb2,
                             bq_sb, bq2_sb, bk_sb, bk2_sb, bv_sb, ones_sb,
                             qTs, kT, vT, outT,
                             wq, wk, wv, wo, bqd, bkd, bvd)
            for _rep in range(repeat):
                _phases(nc, tc, has_bq, has_bk, has_bv, persist_tiles, small)
    _split_multi_waits(nc)
    return nc


def _phases(nc, tc, has_bq, has_bk, has_bv, P, small):
    (khT_pair, khT_h2, qhT_pair, qhT_h2, vhx, wq_sb, wk_sb, wv_sb,
     wo_sb1, wo_sb2, bq_sb, bq2_sb, bk_sb, bk2_sb, bv_sb, ones_sb,
     qTs, kT, vT, outT, wq, wk, wv, wo, bqd, bkd, bvd) = P
    Exp = mybir.ActivationFunctionType.Exp

    def psum_to_sbuf(dst_ap, src_ap, bias_ap):
        if bias_ap is None:
            nc.vector.tensor_copy(dst_ap, src_ap)
        else:
            nc.vector.tensor_scalar_add(dst_ap, src_ap, bias_ap)

    def scores_mms(ps_ap, h, kb, q0, width):
        """scores^T[kb block, q0:q0+width] for head h into PSUM ap."""
        ks = slice(kb * KBLK, (kb + 1) * KBLK)
        if h == 0:
            lhs, rhs = khT_pair[0:64, ks], qhT_pair[0:64, q0:q0 + width]
        elif h == 1:
            lhs, rhs = khT_pair[64:128, ks], qhT_pair[64:128, q0:q0 + width]
        else:
            lhs, rhs = khT_h2[:, ks], qhT_h2[:, q0:q0 + width]
        nc.tensor.matmul(ps_ap, lhs, rhs, start=True, stop=True)

    def normalize_oproj(accs, q0, attnsb, accpool, outsb, tag="acc",
                        tbufs=None):
        attn_pair = attnsb.tile([128, QB], BF16, tag="apair", name="apair")
        attn_h2 = attnsb.tile([64, QB], BF16, tag="ah2", name="ah2")
        for h in range(HPC):
            sums = small.tile([1, QB], FP32, tag="sums", name="sums")
            nc.vector.tensor_copy(sums[:], accs[h][64:65, :])
            rb_ps = accpool.tile([64, QB], FP32, tag=tag, name="rb_ps",
                                 bufs=tbufs)
            nc.tensor.matmul(rb_ps[:], ones_sb[:], sums[:],
                             start=True, stop=True)
            rb = small.tile([64, QB], FP32, tag="rb", name="rb")
            nc.vector.reciprocal(rb[:], rb_ps[:])
            dst = (attn_pair[h * 64:(h + 1) * 64, :]
                   if h < 2 else attn_h2[:])
            nc.vector.tensor_mul(dst, accs[h][0:64, :], rb[:])
            if has_bv:
                nc.vector.tensor_scalar_add(dst, dst, bv_sb[:, h:h + 1])
        for et in range(CT):
            e0 = et * 128
            pso = accpool.tile([128, QB], FP32, tag=tag, name="pso",
                               bufs=tbufs)
            nc.tensor.matmul(pso[:], wo_sb1[:, e0:e0 + 128],
                             attn_pair[:], start=True, stop=False)
            nc.tensor.matmul(pso[:], wo_sb2[:, e0:e0 + 128],
                             attn_h2[:], start=False, stop=True)
            osb = outsb.tile([128, QB], FP32, tag="osb", name="osb")
            nc.vector.tensor_copy(osb[:], pso[:])
            nc.sync.dma_start(outT[e0:e0 + 128, q0:q0 + QB], osb[:])

    # weight loads, ordered to unblock the pipeline front-to-back
    for ct in range(CT):
        nc.sync.dma_start(wq_sb[:, ct * DH:(ct + 1) * DH],
                          wq[ct * 128:ct * 128 + 128, :])
    if has_bq:
        nc.sync.dma_start(bq_sb[:], bqd[0:128, :])
        nc.sync.dma_start(bq2_sb[:], bqd[128:DH, :])

    def load_wkv():
        for ct in range(CT):
            c0 = ct * 128
            nc.sync.dma_start(wk_sb[:, ct * DH:(ct + 1) * DH],
                              wk[c0:c0 + 128, :])
            nc.sync.dma_start(wv_sb[:, ct * DH:(ct + 1) * DH],
                              wv[c0:c0 + 128, :])
        if has_bk:
            nc.sync.dma_start(bk_sb[:], bkd[0:128, :])
            nc.sync.dma_start(bk2_sb[:], bkd[128:DH, :])

    def load_wo():
        nc.sync.dma_start(wo_sb1[:], wo[0:128, :])
        nc.sync.dma_start(wo_sb2[:], wo[128:DH, :])
        if has_bv:
            for h in range(HPC):
                nc.sync.dma_start(bv_sb[:, h:h + 1],
                                  bvd[h * HD:(h + 1) * HD, :])

    # ---- Phase A+B0: projections interleaved with attention for qb 0 ----
    # PSUM budget (8 banks): pk/pk2/pv share a 3-bank projection set,
    # qb0 scores 2 banks, qb0 accumulators 3 banks.
    with (
        tc.tile_pool(name="acc0", bufs=1, space="PSUM") as acc0_pool,
        tc.tile_pool(name="pt0", bufs=4) as pt0_pool,
        tc.tile_pool(name="attnsb", bufs=2) as attnsb,
        tc.tile_pool(name="outsb", bufs=3) as outsb,
      ):
      accs0 = [acc0_pool.tile([128, QB], FP32, tag=f"a0{h}", name="a0",
                              bufs=1)
               for h in range(HPC)]
      with (
        tc.tile_pool(name="stream", bufs=2) as stream,
        tc.tile_pool(name="pproj", bufs=1, space="PSUM") as pproj,
        tc.tile_pool(name="sc0", bufs=2, space="PSUM") as sc0_pool,
      ):
        # q projection (all four query blocks)
        qt2_tiles = []
        for st in range(NQB):
            s0 = st * QB
            ps_q = pproj.tile([128, QB], FP32, tag="pk", name="psq")
            ps_q2 = pproj.tile([64, QB], FP32, tag="pk2", name="psq2")
            if st % 2 == 0:
                qt2_tiles = []
                for ct in range(CT):
                    t = stream.tile([128, 2 * QB], BF16, tag="qt", name="qt",
                                    bufs=12)
                    nc.sync.dma_start(
                        t[:], qTs[ct * 128:(ct + 1) * 128, s0:s0 + 2 * QB])
                    qt2_tiles.append(t)
            qhalf = slice((st % 2) * QB, (st % 2) * QB + QB)
            qt_tiles = [t[:, qhalf] for t in qt2_tiles]
            for ct in range(CT):
                nc.tensor.matmul(
                    ps_q[:], wq_sb[:, ct * DH:ct * DH + 128], qt_tiles[ct][:],
                    start=(ct == 0), stop=(ct == CT - 1))
            for ct in range(CT):
                nc.tensor.matmul(
                    ps_q2[:], wq_sb[:, ct * DH + 128:(ct + 1) * DH],
                    qt_tiles[ct][:],
                    start=(ct == 0), stop=(ct == CT - 1))
            psum_to_sbuf(qhT_pair[:, s0:s0 + QB], ps_q[:],
                         bq_sb[:, 0:1] if has_bq else None)
            psum_to_sbuf(qhT_h2[:, s0:s0 + QB], ps_q2[:],
                         bq2_sb[:, 0:1] if has_bq else None)
            if st == 0:
                load_wkv()

        kt2_tiles = {}
        for kt in range(NKT):
            k0 = kt * KT
            if kt == 2:
                load_wo()
            # k/v loads come in 1024-wide tiles (2KB partition lines);
            # each serves two 512-key superblocks.
            if kt % 2 == 0:
                kw, vw = [], []
                for ct in range(CT):
                    c0 = ct * 128
                    t = stream.tile([128, 2 * KT], BF16, tag="ktile",
                                    name="ktile", bufs=12)
                    nc.sync.dma_start(t[:], kT[c0:c0 + 128, k0:k0 + 2 * KT])
                    kw.append(t)
                    t = stream.tile([128, 2 * KT], BF16, tag="vtile",
                                    name="vtile", bufs=12)
                    nc.sync.dma_start(t[:], vT[c0:c0 + 128, k0:k0 + 2 * KT])
                    vw.append(t)
                kt2_tiles = {"k": kw, "v": vw}
            half = slice((kt % 2) * KT, (kt % 2) * KT + KT)
            kt_tiles = [t[:, half] for t in kt2_tiles["k"]]
            vt_tiles = [t[:, half] for t in kt2_tiles["v"]]
            ps_kh = pproj.tile([128, KT], FP32, tag="pk", name="pskh")
            ps_kh2 = pproj.tile([64, KT], FP32, tag="pk2", name="pskh2")
            for ct in range(CT):
                nc.tensor.matmul(
                    ps_kh[:], wk_sb[:, ct * DH:ct * DH + 128],
                    kt_tiles[ct][:], start=(ct == 0), stop=(ct == CT - 1))
            for ct in range(CT):
                nc.tensor.matmul(
                    ps_kh2[:], wk_sb[:, ct * DH + 128:(ct + 1) * DH],
                    kt_tiles[ct][:], start=(ct == 0), stop=(ct == CT - 1))
            psum_to_sbuf(khT_pair[:, k0:k0 + KT], ps_kh[:],
                         bk_sb[:, 0:1] if has_bk else None)
            psum_to_sbuf(khT_h2[:, k0:k0 + KT], ps_kh2[:],
                         bk2_sb[:, 0:1] if has_bk else None)
            for sj in range(KT // KBLK):
                kb = kt * (KT // KBLK) + sj
                ps_vh = pproj.tile([128, DH], FP32, tag="pv", name="psvh")
                for ct in range(CT):
                    nc.tensor.matmul(
                        ps_vh[:], vt_tiles[ct][:, sj * KBLK:(sj + 1) * KBLK],
                        wv_sb[:, ct * DH:(ct + 1) * DH],
                        start=(ct == 0), stop=(ct == CT - 1))
                for h in range(HPC):
                    nc.vector.tensor_copy(
                        vhx[h][:, kb * 65:kb * 65 + 64],
                        ps_vh[:, h * HD:(h + 1) * HD])
                # attention for query block 0 on this key block
                for h in range(HPC):
                    sc = sc0_pool.tile([128, QB], FP32, tag="sc0", name="sc0")
                    scores_mms(sc[:], h, kb, 0, QB)
                    pt = pt0_pool.tile([128, QB], BF16, tag="pt0", name="pt0")
                    nc.scalar.activation(pt[:], sc[:], Exp, scale=SCALE)
                    nc.tensor.matmul(
                        accs0[h][0:65, :], vhx[h][:, kb * 65:kb * 65 + 65],
                        pt[:], start=(kb == 0), stop=(kb == NKB - 1))
      with tc.tile_pool(name="pfin", bufs=2, space="PSUM") as pfin:
        normalize_oproj(accs0, 0, attnsb, pfin, outsb, tag="fin", tbufs=2)

    # ---- Phase B: attention + o-proj for query blocks 1..3 ----
    with (
        tc.tile_pool(name="scpool", bufs=2, space="PSUM") as scpool,
        tc.tile_pool(name="accpool", bufs=4, space="PSUM") as accpool,
        tc.tile_pool(name="ptpool", bufs=4) as ptpool,
        tc.tile_pool(name="attnsb", bufs=2) as attnsb,
        tc.tile_pool(name="outsb", bufs=3) as outsb,
    ):
        for qb in range(1, NQB):
            q0 = qb * QB
            accs = [accpool.tile([128, QB], FP32, tag="acc", name="acc")
                    for _ in range(HPC)]
            for kb2 in range(NKB // 2):
                pts = []
                for h in range(HPC):
                    ps = scpool.tile([128, 2 * QB], FP32, tag="sc", name="sc")
                    for j in range(2):
                        kb = kb2 * 2 + j
                        scores_mms(ps[:, j * QB:(j + 1) * QB], h, kb, q0, QB)
                    pt = ptpool.tile([128, 2 * QB], BF16, tag="pt", name="pt")
                    nc.scalar.activation(pt[:], ps[:], Exp, scale=SCALE)
                    pts.append(pt)
                for h in range(HPC):
                    for j in range(2):
                        kb = kb2 * 2 + j
                        nc.tensor.matmul(
                            accs[h][0:65, :],
                            vhx[h][:, kb * 65:kb * 65 + 65],
                            pts[h][:, j * QB:(j + 1) * QB],
                            start=(kb == 0), stop=(kb == NKB - 1))
            normalize_oproj(accs, q0, attnsb, accpool, outsb)


def prepare(q, k, v, Wq, bq, Wk, bk, Wv, bv, Wo, bo):
    """Host-side sharding: returns (in_maps for cores 0-7, bias flags)."""
    bf = ml_dtypes.bfloat16
    qT = np.ascontiguousarray(q[0].T).astype(bf)
    kTf = np.ascontiguousarray(k[0].T).astype(bf)
    vTf = np.ascontiguousarray(v[0].T).astype(bf)
    wqT = np.ascontiguousarray(np.asarray(Wq).T).astype(bf)
    wkT = np.ascontiguousarray(np.asarray(Wk).T).astype(bf)
    wvT = np.ascontiguousarray(np.asarray(Wv).T).astype(bf)
    woT = np.ascontiguousarray(np.asarray(Wo).T).astype(bf)
    bq = np.asarray(bq, np.float32)
    bk = np.asarray(bk, np.float32)
    bv = np.asarray(bv, np.float32)
    in_maps = []
    for core in range(8):
        g, s = divmod(core, 2)
        d0, d1 = g * DH, (g + 1) * DH
        in_maps.append({
            "qTs": np.ascontiguousarray(qT[:, s * SQ:(s + 1) * SQ]),
            "kT": kTf,
            "vT": vTf,
            "wq": np.ascontiguousarray(wqT[:, d0:d1]),
            "wk": np.ascontiguousarray(wkT[:, d0:d1]),
            "wv": np.ascontiguousarray(wvT[:, d0:d1]),
            "wo": np.ascontiguousarray(woT[d0:d1, :]),
            "bq": np.ascontiguousarray(bq[d0:d1]).reshape(DH, 1),
            "bk": np.ascontiguousarray(bk[d0:d1]).reshape(DH, 1),
            "bv": np.ascontiguousarray(bv[d0:d1]).reshape(DH, 1),
        })
    flags = (bool(np.any(bq)), bool(np.any(bk)), bool(np.any(bv)))
    return in_maps, flags


def combine(results, bo):
    """Host-side unsharding: sum o-proj partials per half, concat, add bo."""
    halves = []
    for s in range(2):
        acc = None
        for g in range(4):
            o = np.asarray(results[g * 2 + s]["outT"], np.float32)
            acc = o if acc is None else acc + o
        halves.append(acc.T)
    out = np.concatenate(halves, axis=0) + np.asarray(bo, np.float32)
    return np.ascontiguousarray(out).reshape(1, SEQ, D).astype(np.float32)


def kernel(q, k, v, Wq, bq, Wk, bk, Wv, bv, Wo, bo):
    from concourse.bass_utils import run_bass_kernel_spmd

    in_maps, flags = prepare(q, k, v, Wq, bq, Wk, bk, Wv, bv, Wo, bo)
    nc = build_program(*flags)
    last_err = None
    for _attempt in range(3):
        try:
            res = run_bass_kernel_spmd(nc, in_maps, list(range(8)))
            return combine(res.results, bo)
        except Exception as e:  # transient NRT/device wedges recover on retry
            last_err = e
            try:
                import jax
                jax.clear_caches()
                jax.extend.backend.clear_backends()
            except Exception:
                pass
    raise last_err


# revision 24
# speedup vs baseline: 1.1439x; 1.1439x over previous
"""Multi-head attention (B=1, S=4096, D=768, H=12) on 8 Trainium2 NeuronCores.

Sharding: 4 head-groups x 2 sequence-halves. Core (g, s) computes heads
[3g, 3g+3) for query rows [2048*s, 2048*(s+1)): it projects q for its rows,
k/v for its heads over the full sequence, runs softmax(QK^T/8)V for its
(heads, rows) block, and applies its slice of the output projection. The
o-proj partials of the 4 head-groups are summed on the host (the all-reduce
step of tensor-parallel attention), halves concatenated, bias added.

On-chip layout notes:
 - scores are built transposed ([keys, queries]) so the attn@V matmul can
   contract keys on the partition axis with no transposes anywhere.
 - the head pair (h0, h1) shares the 128-row PE array via row tiling
   (K=64 each); the odd head h2 runs in rows 0-63 alone.
 - exp row-sums come for free from the attn@V matmul: V is extended with a
   65th column of ones, so PSUM row 64 accumulates sum_k exp(score).
 - softmax uses no max-subtraction: |scores| < ~30 here, safe in fp32.
"""

import numpy as np
import ml_dtypes

import concourse.bass as bass
import concourse.mybir as mybir
import concourse.tile as tile

BF16 = mybir.dt.bfloat16
FP32 = mybir.dt.float32

D = 768            # model dim
HD = 64            # head dim
HPC = 3            # heads per core
DH = HPC * HD      # 192: head dims per core
SEQ = 4096         # full sequence (keys)
SQ = 2048          # query rows per core
CT = D // 128      # 6 contraction tiles for projections
QB = 512           # query block (matmul free dim)
NQB = SQ // QB     # 4
KBLK = 128         # key block (PSUM partition dim)
NKB = SEQ // KBLK  # 32
KT = 512           # k/v load superblock
NKT = SEQ // KT    # 8
SCALE = 1.0 / 8.0  # 1/sqrt(HD)


def _patch_tile_drain():
    """walrus here accepts only one sync-wait per CTRL instruction; the stock
    TileContext exit packs every outstanding wait onto a single SP Drain.
    Split them onto single-wait SP NOPs that precede the drain."""
    import bass_rust
    from concourse.vector_clock import ScopedClock

    def _split_drain_and_barrier(self, tick_clock, wait_clock):
        nc = self.nc
        probe = nc.sync.nop(nofuse=True)
        wait_clock.add_sem_waits(
            probe.ins, ScopedClock({None: tick_clock.global_clock})
        )
        si = probe.ins.sync_info
        waits = list(si.on_wait) if si is not None and si.on_wait else []
        if len(waits) > 1:
            probe.ins.sync_info = bass_rust.SyncInfo(
                on_wait=[waits[0]], on_update=[]
            )
            for w in waits[1:]:
                n = nc.sync.nop(nofuse=True)
                n.ins.sync_info = bass_rust.SyncInfo(on_wait=[w], on_update=[])
        nc.sync.drain()
        nc.all_engine_barrier()
        assert self.sems is not None
        popped = nc._tile_sem_poison_stack.pop()
        assert popped is self._sem_poison
        nc.clear_and_free_semaphores(list(self.sems.allocated().values()))
        nc.all_engine_barrier()

    tile.TileContext._drain_and_barrier = _split_drain_and_barrier



def _split_multi_waits(nc):
    """Hoist all-but-one sync-waits of every instruction onto preceding
    single-wait NOPs on the same engine (walrus 1-wait limit)."""
    import bass_rust
    n_split = 0
    for bb in nc.main_func.blocks:
        insts = bb.instructions
        new_list = []
        for inst in insts:
            si = getattr(inst, "sync_info", None)
            if si is not None and si.on_wait and len(si.on_wait) > 1:
                waits = list(si.on_wait)
                n_split += 1
                for w in waits[:-1]:
                    nop = mybir.InstNoOp(
                        name=nc.get_next_instruction_name(),
                        engine=inst.engine, ins=[], outs=[],
                        sync_info=bass_rust.SyncInfo(
                            on_wait=[w], on_update=[]))
                    new_list.append(nop)
                inst.sync_info = bass_rust.SyncInfo(
                    on_wait=[waits[-1]], on_update=list(si.on_update))
            new_list.append(inst)
        insts[:] = new_list
    return n_split

def build_program(has_bq: bool, has_bk: bool, has_bv: bool,
                  repeat: int = 1, qk_dtype=BF16) -> bass.Bass:
    _patch_tile_drain()
    nc = bass.Bass()

    qTs = nc.dram_tensor("qTs", [D, SQ], BF16, kind="ExternalInput")
    kT = nc.dram_tensor("kT", [D, SEQ], BF16, kind="ExternalInput")
    vT = nc.dram_tensor("vT", [D, SEQ], BF16, kind="ExternalInput")
    wq = nc.dram_tensor("wq", [D, DH], BF16, kind="ExternalInput")
    wk = nc.dram_tensor("wk", [D, DH], BF16, kind="ExternalInput")
    wv = nc.dram_tensor("wv", [D, DH], BF16, kind="ExternalInput")
    wo = nc.dram_tensor("wo", [DH, D], BF16, kind="ExternalInput")
    bqd = nc.dram_tensor("bq", [DH, 1], FP32, kind="ExternalInput")
    bkd = nc.dram_tensor("bk", [DH, 1], FP32, kind="ExternalInput")
    bvd = nc.dram_tensor("bv", [DH, 1], FP32, kind="ExternalInput")
    outT = nc.dram_tensor("outT", [D, SQ], FP32, kind="ExternalOutput")

    Exp = mybir.ActivationFunctionType.Exp

    with tile.TileContext(nc) as tc:
        with (
            tc.tile_pool(name="persist", bufs=1) as persist,
            tc.tile_pool(name="small", bufs=2) as small,
        ):
            # persistent SBUF tensors
            khT_pair = persist.tile([128, SEQ], qk_dtype, tag="khp", name="khp")
            khT_h2 = persist.tile([64, SEQ], qk_dtype, tag="kh2", name="kh2")
            qhT_pair = persist.tile([128, SQ], qk_dtype, tag="qhp", name="qhp")
            qhT_h2 = persist.tile([64, SQ], qk_dtype, tag="qh2", name="qh2")
            vhx = [persist.tile([128, NKB * 65], BF16, tag=f"vhx{h}", name=f"vhx{h}")
                   for h in range(HPC)]
            wq_sb = persist.tile([128, CT * DH], BF16, tag="wq", name="wq_sb")
            wk_sb = persist.tile([128, CT * DH], BF16, tag="wk", name="wk_sb")
            wv_sb = persist.tile([128, CT * DH], BF16, tag="wv", name="wv_sb")
            wo_sb1 = persist.tile([128, D], BF16, tag="wo1", name="wo1")
            wo_sb2 = persist.tile([64, D], BF16, tag="wo2", name="wo2")
            bq_sb = persist.tile([128, 1], FP32, tag="bq1", name="bq1")
            bq2_sb = persist.tile([64, 1], FP32, tag="bq2", name="bq2")
            bk_sb = persist.tile([128, 1], FP32, tag="bk1", name="bk1")
            bk2_sb = persist.tile([64, 1], FP32, tag="bk2", name="bk2")
            bv_sb = persist.tile([64, HPC], FP32, tag="bv", name="bv_sb")
            ones_sb = persist.tile([1, 64], FP32, tag="ones", name="ones_sb")

            # ones columns for the exp-sum trick (overwritten with vh below)
            for h in range(HPC):
                nc.gpsimd.memset(vhx[h][:], 1.0)
            nc.vector.memset(ones_sb[:], 1.0)

            persist_tiles = (khT_pair, khT_h2, qhT_pair, qhT_h2, vhx,
                             wq_sb, wk_sb, wv_sb, wo_sb1, wo_sb2,
                             bq_sb, bq2_sb, bk_sb, bk2_sb, bv_sb, ones_sb,
                             qTs, kT, vT, outT,
                             wq, wk, wv, wo, bqd, bkd, bvd)
            for _rep in range(repeat):
                _phases(nc, tc, has_bq, has_bk, has_bv, persist_tiles, small)
    _split_multi_waits(nc)
    return nc


def _phases(nc, tc, has_bq, has_bk, has_bv, P, small):
    (khT_pair, khT_h2, qhT_pair, qhT_h2, vhx, wq_sb, wk_sb, wv_sb,
     wo_sb1, wo_sb2, bq_sb, bq2_sb, bk_sb, bk2_sb, bv_sb, ones_sb,
     qTs, kT, vT, outT, wq, wk, wv, wo, bqd, bkd, bvd) = P
    Exp = mybir.ActivationFunctionType.Exp

    def psum_to_sbuf(dst_ap, src_ap, bias_ap):
        if bias_ap is None:
            nc.vector.tensor_copy(dst_ap, src_ap)
        else:
            nc.vector.tensor_scalar_add(dst_ap, src_ap, bias_ap)

    def scores_mms(ps_ap, h, kb, q0, width):
        """scores^T[kb block, q0:q0+width] for head h into PSUM ap."""
        ks = slice(kb * KBLK, (kb + 1) * KBLK)
        if h == 0:
            lhs, rhs = khT_pair[0:64, ks], qhT_pair[0:64, q0:q0 + width]
        elif h == 1:
            lhs, rhs = khT_pair[64:128, ks], qhT_pair[64:128, q0:q0 + width]
        else:
            lhs, rhs = khT_h2[:, ks], qhT_h2[:, q0:q0 + width]
        nc.tensor.matmul(ps_ap, lhs, rhs, start=True, stop=True)

    def normalize_oproj(accs, q0, attnsb, accpool, outsb, tag="acc",
                        tbufs=None):
        attn_pair = attnsb.tile([128, QB], BF16, tag="apair", name="apair")
        attn_h2 = attnsb.tile([64, QB], BF16, tag="ah2", name="ah2")
        for h in range(HPC):
            sums = small.tile([1, QB], FP32, tag="sums", name="sums")
            nc.vector.tensor_copy(sums[:], accs[h][64:65, :])
            rb_ps = accpool.tile([64, QB], FP32, tag=tag, name="rb_ps",
                                 bufs=tbufs)
            nc.tensor.matmul(rb_ps[:], ones_sb[:], sums[:],
                             start=True, stop=True)
            rb = small.tile([64, QB], FP32, tag="rb", name="rb")
            nc.vector.reciprocal(rb[:], rb_ps[:])
            dst = (attn_pair[h * 64:(h + 1) * 64, :]
                   if h < 2 else attn_h2[:])
            nc.vector.tensor_mul(dst, accs[h][0:64, :], rb[:])
            if has_bv:
                nc.vector.tensor_scalar_add(dst, dst, bv_sb[:, h:h + 1])
        for et in range(CT):
            e0 = et * 128
            pso = accpool.tile([128, QB], FP32, tag=tag, name="pso",
                               bufs=tbufs)
            nc.tensor.matmul(pso[:], wo_sb1[:, e0:e0 + 128],
                             attn_pair[:], start=True, stop=False)
            nc.tensor.matmul(pso[:], wo_sb2[:, e0:e0 + 128],
                             attn_h2[:], start=False, stop=True)
            osb = outsb.tile([128, QB], FP32, tag="osb", name="osb")
            nc.vector.tensor_copy(osb[:], pso[:])
            nc.sync.dma_start(outT[e0:e0 + 128, q0:q0 + QB], osb[:])

    # weight loads, ordered to unblock the pipeline front-to-back
    for ct in range(CT):
        nc.sync.dma_start(wq_sb[:, ct * DH:(ct + 1) * DH],
                          wq[ct * 128:ct * 128 + 128, :])
    if has_bq:
        nc.sync.dma_start(bq_sb[:], bqd[0:128, :])
        nc.sync.dma_start(bq2_sb[:], bqd[128:DH, :])

    def load_wkv():
        for ct in range(CT):
            c0 = ct * 128
            nc.sync.dma_start(wk_sb[:, ct * DH:(ct + 1) * DH],
                              wk[c0:c0 + 128, :])
            nc.sync.dma_start(wv_sb[:, ct * DH:(ct + 1) * DH],
                              wv[c0:c0 + 128, :])
        if has_bk:
            nc.sync.dma_start(bk_sb[:], bkd[0:128, :])
            nc.sync.dma_start(bk2_sb[:], bkd[128:DH, :])

    def load_wo():
        nc.sync.dma_start(wo_sb1[:], wo[0:128, :])
        nc.sync.dma_start(wo_sb2[:], wo[128:DH, :])
        if has_bv:
            for h in range(HPC):
                nc.sync.dma_start(bv_sb[:, h:h + 1],
                                  bvd[h * HD:(h + 1) * HD, :])

    # ---- Phase A+B0: projections interleaved with attention for qb 0 ----
    # PSUM budget (8 banks): pk/pk2/pv share a 3-bank projection set,
    # qb0 scores 2 banks, qb0 accumulators 3 banks.
    with (
        tc.tile_pool(name="acc0", bufs=1, space="PSUM") as acc0_pool,
        tc.tile_pool(name="pt0", bufs=6) as pt0_pool,
        tc.tile_pool(name="attnsb", bufs=2) as attnsb,
        tc.tile_pool(name="outsb", bufs=3) as outsb,
      ):
      accs0 = [acc0_pool.tile([128, QB], FP32, tag=f"a0{h}", name="a0",
                              bufs=1)
               for h in range(HPC)]
      with (
        tc.tile_pool(name="stream", bufs=2) as stream,
        tc.tile_pool(name="pproj", bufs=1, space="PSUM") as pproj,
        tc.tile_pool(name="sc0", bufs=2, space="PSUM") as sc0_pool,
      ):
        # q projection (all four query blocks)
        qt2_tiles = []
        for st in range(NQB):
            s0 = st * QB
            ps_q = pproj.tile([128, QB], FP32, tag="pk", name="psq")
            ps_q2 = pproj.tile([64, QB], FP32, tag="pk2", name="psq2")
            if st % 2 == 0:
                qt2_tiles = []
                for ct in range(CT):
                    t = stream.tile([128, 2 * QB], BF16, tag="qt", name="qt",
                                    bufs=12)
                    nc.sync.dma_start(
                        t[:], qTs[ct * 128:(ct + 1) * 128, s0:s0 + 2 * QB])
                    qt2_tiles.append(t)
            qhalf = slice((st % 2) * QB, (st % 2) * QB + QB)
            qt_tiles = [t[:, qhalf] for t in qt2_tiles]
            for ct in range(CT):
                nc.tensor.matmul(
                    ps_q[:], wq_sb[:, ct * DH:ct * DH + 128], qt_tiles[ct][:],
                    start=(ct == 0), stop=(ct == CT - 1))
            for ct in range(CT):
                nc.tensor.matmul(
                    ps_q2[:], wq_sb[:, ct * DH + 128:(ct + 1) * DH],
                    qt_tiles[ct][:],
                    start=(ct == 0), stop=(ct == CT - 1))
            psum_to_sbuf(qhT_pair[:, s0:s0 + QB], ps_q[:],
                         bq_sb[:, 0:1] if has_bq else None)
            psum_to_sbuf(qhT_h2[:, s0:s0 + QB], ps_q2[:],
                         bq2_sb[:, 0:1] if has_bq else None)
            if st == 0:
                load_wkv()

        kt2_tiles = {}
        for kt in range(NKT):
            k0 = kt * KT
            if kt == 2:
                load_wo()
            # k/v loads come in 1024-wide tiles (2KB partition lines);
            # each serves two 512-key superblocks.
            if kt % 2 == 0:
                kw, vw = [], []
                for ct in range(CT):
                    c0 = ct * 128
                    t = stream.tile([128, 2 * KT], BF16, tag="ktile",
                                    name="ktile", bufs=12)
                    nc.sync.dma_start(t[:], kT[c0:c0 + 128, k0:k0 + 2 * KT])
                    kw.append(t)
                    t = stream.tile([128, 2 * KT], BF16, tag="vtile",
                                    name="vtile", bufs=12)
                    nc.sync.dma_start(t[:], vT[c0:c0 + 128, k0:k0 + 2 * KT])
                    vw.append(t)
                kt2_tiles = {"k": kw, "v": vw}
            half = slice((kt % 2) * KT, (kt % 2) * KT + KT)
            kt_tiles = [t[:, half] for t in kt2_tiles["k"]]
            vt_tiles = [t[:, half] for t in kt2_tiles["v"]]
            ps_kh = pproj.tile([128, KT], FP32, tag="pk", name="pskh")
            ps_kh2 = pproj.tile([64, KT], FP32, tag="pk2", name="pskh2")
            for ct in range(CT):
                nc.tensor.matmul(
                    ps_kh[:], wk_sb[:, ct * DH:ct * DH + 128],
                    kt_tiles[ct][:], start=(ct == 0), stop=(ct == CT - 1))
            for ct in range(CT):
                nc.tensor.matmul(
                    ps_kh2[:], wk_sb[:, ct * DH + 128:(ct + 1) * DH],
                    kt_tiles[ct][:], start=(ct == 0), stop=(ct == CT - 1))
            psum_to_sbuf(khT_pair[:, k0:k0 + KT], ps_kh[:],
                         bk_sb[:, 0:1] if has_bk else None)
            psum_to_sbuf(khT_h2[:, k0:k0 + KT], ps_kh2[:],
                         bk2_sb[:, 0:1] if has_bk else None)
            for sj in range(KT // KBLK):
                kb = kt * (KT // KBLK) + sj
                ps_vh = pproj.tile([128, DH], FP32, tag="pv", name="psvh")
                for ct in range(CT):
                    nc.tensor.matmul(
                        ps_vh[:], vt_tiles[ct][:, sj * KBLK:(sj + 1) * KBLK],
                        wv_sb[:, ct * DH:(ct + 1) * DH],
                        start=(ct == 0), stop=(ct == CT - 1))
                for h in range(HPC):
                    nc.vector.tensor_copy(
                        vhx[h][:, kb * 65:kb * 65 + 64],
                        ps_vh[:, h * HD:(h + 1) * HD])
                # attention for query block 0 on this key block
                for h in range(HPC):
                    sc = sc0_pool.tile([128, QB], FP32, tag="sc0", name="sc0")
                    scores_mms(sc[:], h, kb, 0, QB)
                    pt = pt0_pool.tile([128, QB], BF16, tag="pt0", name="pt0")
                    nc.scalar.activation(pt[:], sc[:], Exp, scale=SCALE)
                    nc.tensor.matmul(
                        accs0[h][0:65, :], vhx[h][:, kb * 65:kb * 65 + 65],
                        pt[:], start=(kb == 0), stop=(kb == NKB - 1))
      with tc.tile_pool(name="pfin", bufs=2, space="PSUM") as pfin:
        normalize_oproj(accs0, 0, attnsb, pfin, outsb, tag="fin", tbufs=2)

    # ---- Phase B: attention + o-proj for query blocks 1..3 ----
    with (
        tc.tile_pool(name="scpool", bufs=2, space="PSUM") as scpool,
        tc.tile_pool(name="accpool", bufs=4, space="PSUM") as accpool,
        tc.tile_pool(name="ptpool", bufs=8) as ptpool,
        tc.tile_pool(name="attnsb", bufs=2) as attnsb,
        tc.tile_pool(name="outsb", bufs=3) as outsb,
    ):
        for qb in range(1, NQB):
            q0 = qb * QB
            accs = [accpool.tile([128, QB], FP32, tag="acc", name="acc")
                    for _ in range(HPC)]
            for kb2 in range(NKB // 2):
                pts = []
                for h in range(HPC):
                    ps = scpool.tile([128, 2 * QB], FP32, tag="sc", name="sc")
                    for j in range(2):
                        kb = kb2 * 2 + j
                        scores_mms(ps[:, j * QB:(j + 1) * QB], h, kb, q0, QB)
                    pt = ptpool.tile([128, 2 * QB], BF16, tag="pt", name="pt")
                    nc.scalar.activation(pt[:], ps[:], Exp, scale=SCALE)
                    pts.append(pt)
                for h in range(HPC):
                    for j in range(2):
                        kb = kb2 * 2 + j
                        nc.tensor.matmul(
                            accs[h][0:65, :],
                            vhx[h][:, kb * 65:kb * 65 + 65],
                            pts[h][:, j * QB:(j + 1) * QB],
                            start=(kb == 0), stop=(kb == NKB - 1))
            normalize_oproj(accs, q0, attnsb, accpool, outsb)


def prepare(q, k, v, Wq, bq, Wk, bk, Wv, bv, Wo, bo):
    """Host-side sharding: returns (in_maps for cores 0-7, bias flags)."""
    bf = ml_dtypes.bfloat16
    qT = np.ascontiguousarray(q[0].T).astype(bf)
    kTf = np.ascontiguousarray(k[0].T).astype(bf)
    vTf = np.ascontiguousarray(v[0].T).astype(bf)
    wqT = np.ascontiguousarray(np.asarray(Wq).T).astype(bf)
    wkT = np.ascontiguousarray(np.asarray(Wk).T).astype(bf)
    wvT = np.ascontiguousarray(np.asarray(Wv).T).astype(bf)
    woT = np.ascontiguousarray(np.asarray(Wo).T).astype(bf)
    bq = np.asarray(bq, np.float32)
    bk = np.asarray(bk, np.float32)
    bv = np.asarray(bv, np.float32)
    in_maps = []
    for core in range(8):
        g, s = divmod(core, 2)
        d0, d1 = g * DH, (g + 1) * DH
        in_maps.append({
            "qTs": np.ascontiguousarray(qT[:, s * SQ:(s + 1) * SQ]),
            "kT": kTf,
            "vT": vTf,
            "wq": np.ascontiguousarray(wqT[:, d0:d1]),
            "wk": np.ascontiguousarray(wkT[:, d0:d1]),
            "wv": np.ascontiguousarray(wvT[:, d0:d1]),
            "wo": np.ascontiguousarray(woT[d0:d1, :]),
            "bq": np.ascontiguousarray(bq[d0:d1]).reshape(DH, 1),
            "bk": np.ascontiguousarray(bk[d0:d1]).reshape(DH, 1),
            "bv": np.ascontiguousarray(bv[d0:d1]).reshape(DH, 1),
        })
    flags = (bool(np.any(bq)), bool(np.any(bk)), bool(np.any(bv)))
    return in_maps, flags


def combine(results, bo):
    """Host-side unsharding: sum o-proj partials per half, concat, add bo."""
    halves = []
    for s in range(2):
        acc = None
        for g in range(4):
            o = np.asarray(results[g * 2 + s]["outT"], np.float32)
            acc = o if acc is None else acc + o
        halves.append(acc.T)
    out = np.concatenate(halves, axis=0) + np.asarray(bo, np.float32)
    return np.ascontiguousarray(out).reshape(1, SEQ, D).astype(np.float32)


def kernel(q, k, v, Wq, bq, Wk, bk, Wv, bv, Wo, bo):
    from concourse.bass_utils import run_bass_kernel_spmd

    in_maps, flags = prepare(q, k, v, Wq, bq, Wk, bk, Wv, bv, Wo, bo)
    nc = build_program(*flags)
    last_err = None
    for _attempt in range(3):
        try:
            res = run_bass_kernel_spmd(nc, in_maps, list(range(8)))
            return combine(res.results, bo)
        except Exception as e:  # transient NRT/device wedges recover on retry
            last_err = e
            try:
                import jax
                jax.clear_caches()
                jax.extend.backend.clear_backends()
            except Exception:
                pass
    raise last_err
